# revision 10
# baseline (speedup 1.0000x reference)
"""Multi-head attention with RoPE (B=4, S=2048, D=1024, H=16, hd=64), causal.

Sharding: 8 cores = 4 batches x 2 head-groups. Core c handles batch c//2,
heads 8*(c%2)..8*(c%2)+8 (columns 512*(c%2)..+512 of Wq/Wk/Wv, rows of Wo).
Each core computes a partial output [S, D] (its head-group's contribution
through Wo); host sums the two partials per batch and adds bo.

Per-core kernel (all matmuls in fp32r = full-rate fp32-relaxed):
  - X^T built via PE transposes (contraction over D needs D on partitions)
  - Q^T/K^T computed in transposed layout [cols, S] with host-permuted
    weight columns (all even RoPE pair-members first, then all odd), so RoPE
    is full-tile elementwise work; V in natural [S, cols] layout
  - scores^T = K'Q'^T per head via 4x row-tiled (K=32) matmuls, evens+odds
    accumulated in PSUM; exp on ACT (no max subtraction needed: |score|<~8);
    causal masking via gpsimd affine_select zero-fill on straddle tiles only
  - PV: lhsT=[V_h | ones] (M=65) so row 64 accumulates the softmax
    denominator in the same PSUM stream; normalize with DVE reciprocal +
    K=1 ones-matmul partition-broadcast
  - output projection from the transposed attention output
"""

import numpy as np

import concourse.bass as bass
import concourse.mybir as mybir
import concourse.tile as tile
from concourse import bacc
from concourse.bass_utils import run_bass_kernel_spmd

P = 128
D = 1024
COLS = 512  # per-core Q/K/V columns (8 heads * 64)
HD = 64
NHL = 8  # local heads per core
ROPE_BASE = 10000.0
F32 = mybir.dt.float32
F32R = mybir.dt.float32r

_CACHE = {}


def build_kernel(S):
    n_sc = S // 512  # s-chunks (projection) and q-blocks (attention)
    nc = bacc.Bacc("TRN2", target_bir_lowering=False, debug=False, num_devices=8)

    x_h = nc.dram_tensor("x", [S, D], F32, kind="ExternalInput").ap()
    wq_h = nc.dram_tensor("wq", [D, COLS], F32, kind="ExternalInput").ap()
    wk_h = nc.dram_tensor("wk", [D, COLS], F32, kind="ExternalInput").ap()
    wv_h = nc.dram_tensor("wv", [D, COLS], F32, kind="ExternalInput").ap()
    wo_h = nc.dram_tensor("wo", [COLS, D], F32, kind="ExternalInput").ap()
    bq_h = nc.dram_tensor("bq", [P, 4], F32, kind="ExternalInput").ap()
    bk_h = nc.dram_tensor("bk", [P, 4], F32, kind="ExternalInput").ap()
    bv_h = nc.dram_tensor("bv", [1, COLS], F32, kind="ExternalInput").ap()
    cos_h = nc.dram_tensor("cosv", [256, S], F32, kind="ExternalInput").ap()
    sin_h = nc.dram_tensor("sinv", [256, S], F32, kind="ExternalInput").ap()
    o_h = nc.dram_tensor("o", [S, D], F32, kind="ExternalOutput").ap()
    scr_h = nc.dram_tensor("attnT_scr", [4, S // P, P, P], F32R).ap()

    with tile.TileContext(nc) as tc:
        with (
            tc.tile_pool(name="const", bufs=1) as constp,
            tc.tile_pool(name="qk", bufs=1) as qkp,
            tc.tile_pool(name="vp", bufs=1) as vp,
        ):
            ident = constp.tile([P, P], F32R)
            nc.gpsimd.memset(ident.bitcast(F32), 0.0)
            nc.gpsimd.affine_select(
                out=ident, in_=ident, compare_op=mybir.AluOpType.not_equal,
                fill=1.0, base=0, pattern=[[-1, P]], channel_multiplier=1,
            )
            ones1x128 = constp.tile([1, P], F32R)
            nc.vector.memset(ones1x128.bitcast(F32), 1.0)
            bq_sb = constp.tile([P, 4], F32)
            nc.sync.dma_start(out=bq_sb, in_=bq_h)
            bk_sb = constp.tile([P, 4], F32)
            nc.sync.dma_start(out=bk_sb, in_=bk_h)
            bv_sb = constp.tile([1, COLS], F32R)
            nc.sync.dma_start(out=bv_sb, in_=bv_h.bitcast(F32R))

            # persistent activations
            qt = [qkp.tile([P, S], F32R, name=f"qt{i}") for i in range(4)]
            kt_ = [qkp.tile([P, S], F32R, name=f"kt{i}") for i in range(4)]
            # V: [128 keys, 8 heads, 65] per s-tile; col 64 = ones (denominator)
            vt_all = vp.tile([P, S // P, NHL, HD + 1], F32R, name="vt_all")

            # ---------------- Phase A: transpose + projections + RoPE ------
            with (
                tc.tile_pool(name="wqkv", bufs=1) as wp,
                tc.tile_pool(name="xa", bufs=2) as xp,
                tc.tile_pool(name="xta", bufs=1) as xtp,
                tc.tile_pool(name="cs", bufs=1) as csp,
                tc.tile_pool(name="tmp", bufs=2) as tmpp,
                tc.tile_pool(name="psA", bufs=4, space="PSUM") as psA,
                tc.tile_pool(name="psT", bufs=2, space="PSUM") as psT,
            ):
                wq_sb = wp.tile([P, 8, COLS], F32R, name="wq_sb")
                nc.sync.dma_start(
                    out=wq_sb, in_=wq_h.rearrange("(kd p) c -> p kd c", p=P).bitcast(F32R)
                )
                wk_sb = wp.tile([P, 8, COLS], F32R, name="wk_sb")
                nc.sync.dma_start(
                    out=wk_sb, in_=wk_h.rearrange("(kd p) c -> p kd c", p=P).bitcast(F32R)
                )
                wv_sb = wp.tile([P, 8, COLS], F32R, name="wv_sb")
                nc.sync.dma_start(
                    out=wv_sb, in_=wv_h.rearrange("(kd p) c -> p kd c", p=P).bitcast(F32R)
                )
                nc.vector.memset(vt_all[:, :, :, HD : HD + 1].bitcast(F32), 1.0)

                for sc in range(n_sc):
                    cos_sb = csp.tile([P, 2, 512], F32R, name="cos_sb")
                    nc.sync.dma_start(
                        out=cos_sb,
                        in_=cos_h.rearrange("(h p) s -> p h s", p=P)[
                            :, :, sc * 512 : (sc + 1) * 512
                        ].bitcast(F32R),
                    )
                    sin_sb = csp.tile([P, 2, 512], F32R, name="sin_sb")
                    nc.sync.dma_start(
                        out=sin_sb,
                        in_=sin_h.rearrange("(h p) s -> p h s", p=P)[
                            :, :, sc * 512 : (sc + 1) * 512
                        ].bitcast(F32R),
                    )

                    # X^T chunk: [128 (d in kd), 8 kd, 512 s-local]
                    xt_sb = xtp.tile([P, 8, 512], F32R, name="xt_sb")
                    for si in range(4):
                        x_sb = xp.tile([P, D], F32R, name="x_sb")
                        nc.sync.dma_start(
                            out=x_sb,
                            in_=x_h[sc * 512 + si * P : sc * 512 + (si + 1) * P, :]
                            .bitcast(F32R),
                        )
                        for kd in range(8):
                            pt = psT.tile([P, P], F32R, name="pt")
                            nc.tensor.transpose(
                                pt[:], x_sb[:, kd * P : (kd + 1) * P], ident[:]
                            )
                            eng = nc.vector if (kd + si) % 2 == 0 else nc.scalar
                            if eng is nc.vector:
                                nc.vector.tensor_copy(
                                    xt_sb[:, kd, si * P : (si + 1) * P], pt[:]
                                )
                            else:
                                nc.scalar.copy(
                                    xt_sb[:, kd, si * P : (si + 1) * P], pt[:]
                                )

                    # V chunk: natural layout, + bias via K=1 matmul
                    for si in range(4):
                        pv = psA.tile([P, COLS], F32, name="pv", tag="ps")
                        for kd in range(8):
                            nc.tensor.matmul(
                                pv[:],
                                xt_sb[:, kd, si * P : (si + 1) * P],
                                wv_sb[:, kd, :],
                                start=(kd == 0),
                                stop=False,
                            )
                        nc.tensor.matmul(
                            pv[:], ones1x128[:, :], bv_sb[:, :], start=False, stop=True
                        )
                        nc.scalar.copy(
                            vt_all[:, sc * 4 + si, :, 0:HD],
                            pv.rearrange("p (h e) -> p h e", h=NHL),
                        )

                    # Q^T, K^T raw chunks (permuted cols), bias per partition
                    for name, w_sb, b_sb, dst in (
                        ("q", wq_sb, bq_sb, qt),
                        ("k", wk_sb, bk_sb, kt_),
                    ):
                        for mc in range(4):
                            pq = psA.tile([P, 512], F32, name="pq", tag="ps")
                            for kd in range(8):
                                nc.tensor.matmul(
                                    pq[:],
                                    w_sb[:, kd, mc * P : (mc + 1) * P],
                                    xt_sb[:, kd, :],
                                    start=(kd == 0),
                                    stop=(kd == 7),
                                )
                            nc.scalar.activation(
                                dst[mc][:, sc * 512 : (sc + 1) * 512],
                                pq[:],
                                mybir.ActivationFunctionType.Identity,
                                bias=b_sb[:, mc : mc + 1],
                            )

                    # RoPE in place: x1 = tiles 0,1 (evens), x2 = tiles 2,3 (odds)
                    ssl = slice(sc * 512, (sc + 1) * 512)
                    for dst in (qt, kt_):
                        for mc in range(2):
                            x1 = dst[mc][:, ssl]
                            x2 = dst[mc + 2][:, ssl]
                            c = cos_sb[:, mc, :]
                            s_ = sin_sb[:, mc, :]
                            t4 = tmpp.tile([P, 4, 512], F32R, name="t4")
                            nc.vector.tensor_mul(t4[:, 0, :], x1, c)
                            nc.vector.tensor_mul(t4[:, 1, :], x2, s_)
                            nc.vector.tensor_mul(t4[:, 2, :], x1, s_)
                            nc.vector.tensor_mul(t4[:, 3, :], x2, c)
                            nc.gpsimd.tensor_sub(x1, t4[:, 0, :], t4[:, 1, :])
                            nc.gpsimd.tensor_add(x2, t4[:, 2, :], t4[:, 3, :])

            # ---------------- Phase B: attention --------------------------
            with tc.tile_pool(name="wo", bufs=1) as wop:
                wo_sb = wop.tile([P, 4, D], F32R, name="wo_sb")
                nc.sync.dma_start(
                    out=wo_sb, in_=wo_h.rearrange("(kc p) n -> p kc n", p=P).bitcast(F32R)
                )

                with (
                    tc.tile_pool(name="ep", bufs=3) as ep,
                    tc.tile_pool(name="up", bufs=2) as up,
                    tc.tile_pool(name="rp", bufs=2) as rp,
                    tc.tile_pool(name="psS", bufs=4, space="PSUM") as psS,
                    tc.tile_pool(name="psP", bufs=4, space="PSUM") as psP,
                ):
                    for qb in range(n_sc):
                        nk = 4 * qb + 4
                        qsl = slice(qb * 512, (qb + 1) * 512)
                        for Q in range(2):
                            pb = [
                                psP.tile([HD + 1, 512], F32, name="pb", tag="pb")
                                for _ in range(4)
                            ]
                            for kt in range(nk):
                                e4 = ep.tile([P, 4, 512], F32R, name="e4", tag="e")
                                sp_tiles = []
                                for hi in range(4):
                                    rg = 32 * hi
                                    sp = psS.tile([P, 512], F32, name="sp", tag="sp")
                                    nc.tensor.matmul(
                                        sp[:],
                                        kt_[Q][rg : rg + 32, kt * P : (kt + 1) * P],
                                        qt[Q][rg : rg + 32, qsl],
                                        start=True,
                                        stop=False,
                                        tile_position=(rg, 0),
                                    )
                                    nc.tensor.matmul(
                                        sp[:],
                                        kt_[Q + 2][rg : rg + 32, kt * P : (kt + 1) * P],
                                        qt[Q + 2][rg : rg + 32, qsl],
                                        start=False,
                                        stop=True,
                                        tile_position=(rg, 0),
                                    )
                                    sp_tiles.append(sp)
                                for hi in range(4):
                                    hl = 4 * Q + hi
                                    e = e4[:, hi, :]
                                    nc.scalar.activation(
                                        e,
                                        sp_tiles[hi][:],
                                        mybir.ActivationFunctionType.Exp,
                                        scale=0.125,
                                    )
                                    w = kt * P + P - qb * 512
                                    if 0 < w <= 512:  # straddle: zero where q < k
                                        nc.gpsimd.affine_select(
                                            out=e[:, 0:w],
                                            in_=e[:, 0:w],
                                            compare_op=mybir.AluOpType.is_ge,
                                            fill=0.0,
                                            base=qb * 512 - kt * P,
                                            pattern=[[1, w]],
                                            channel_multiplier=-1,
                                        )
                                    nc.tensor.matmul(
                                        pb[hi][:],
                                        vt_all[:, kt, hl, :],
                                        e,
                                        start=(kt == 0),
                                        stop=(kt == nk - 1),
                                    )
                            for hi in range(4):
                                hl = 4 * Q + hi
                                rec = rp.tile([1, 512], F32R, name="rec", tag="rec")
                                with nc.allow_low_precision(reason="f32r is fp32"):
                                    nc.vector.reciprocal(rec, pb[hi][HD : HD + 1, :])
                                bc = psS.tile([HD, 512], F32, name="bc", tag="sp")
                                nc.tensor.matmul(
                                    bc[:],
                                    ones1x128[:, 0:HD],
                                    rec[:],
                                    start=True,
                                    stop=True,
                                )
                                u = up.tile([HD, 512], F32R, name="u", tag="u")
                                nc.scalar.copy(u, pb[hi][0:HD, :])
                                an = up.tile([HD, 512], F32R, name="an", tag="an")
                                nc.vector.tensor_mul(an, u, bc[:])
                                nc.sync.dma_start(
                                    out=scr_h[
                                        hl // 2, qb * 4 : (qb + 1) * 4,
                                        64 * (hl % 2) : 64 * (hl % 2) + 64, :,
                                    ].rearrange("st p c -> p st c"),
                                    in_=an.rearrange("p (st c) -> p st c", c=P),
                                )

                # ---------------- Phase C: output projection ---------------
                with (
                    tc.tile_pool(name="outp", bufs=4) as outp,
                    tc.tile_pool(name="atc", bufs=4) as atcp,
                    tc.tile_pool(name="psO", bufs=4, space="PSUM") as psO,
                ):
                    for st in range(S // P):
                        atc = atcp.tile([P, 4, P], F32R, name="atc")
                        nc.sync.dma_start(
                            out=atc,
                            in_=scr_h[:, st, :, :].rearrange("k p c -> p k c"),
                        )
                        for nchunk in range(2):
                            po = psO.tile([P, 512], F32, name="po")
                            for kc in range(4):
                                nc.tensor.matmul(
                                    po[:],
                                    atc[:, kc, :],
                                    wo_sb[:, kc, nchunk * 512 : (nchunk + 1) * 512],
                                    start=(kc == 0),
                                    stop=(kc == 3),
                                )
                            ot = outp.tile([P, 512], F32, name="ot")
                            if (st + nchunk) % 2 == 0:
                                nc.vector.tensor_copy(ot, po[:])
                            else:
                                nc.scalar.copy(ot, po[:])
                            nc.sync.dma_start(
                                out=o_h[
                                    st * P : (st + 1) * P,
                                    nchunk * 512 : (nchunk + 1) * 512,
                                ],
                                in_=ot,
                            )

    nc.compile()
    return nc


def _perm512():
    """Column permutation: all even pair-members first, then all odd."""
    idx = np.arange(COLS)
    return np.concatenate([idx[0::2], idx[1::2]])


def prep_core_inputs(X, Wq, bq, Wk, bk, Wv, bv, Wo, bo, S):
    """Build the 8 per-core input maps (host-side sharding)."""
    perm = _perm512()
    half = D // 2
    inv_freq = 1.0 / (ROPE_BASE ** (np.arange(half, dtype=np.float64) / half))
    pos = np.arange(S, dtype=np.float64)
    maps = []
    for c in range(8):
        b = c // 2
        g = c % 2
        c0 = COLS * g
        sl = slice(c0, c0 + COLS)
        # frequencies for this head-group's 256 pairs
        fr = inv_freq[c0 // 2 : c0 // 2 + 256]
        ang = pos[None, :] * fr[:, None]  # [256, S]
        maps.append(
            {
                "x": np.ascontiguousarray(X[b]).astype(np.float32),
                "wq": np.ascontiguousarray(Wq[:, sl][:, perm]).astype(np.float32),
                "wk": np.ascontiguousarray(Wk[:, sl][:, perm]).astype(np.float32),
                "wv": np.ascontiguousarray(Wv[:, sl]).astype(np.float32),
                "wo": np.ascontiguousarray(Wo[sl, :]).astype(np.float32),
                "bq": np.ascontiguousarray(
                    bq[sl][perm].reshape(4, P).T
                ).astype(np.float32),
                "bk": np.ascontiguousarray(
                    bk[sl][perm].reshape(4, P).T
                ).astype(np.float32),
                "bv": np.ascontiguousarray(bv[sl].reshape(1, COLS)).astype(np.float32),
                "cosv": np.cos(ang).astype(np.float32),
                "sinv": np.sin(ang).astype(np.float32),
            }
        )
    return maps


def kernel(X, Wq, bq, Wk, bk, Wv, bv, Wo, bo):
    X = np.asarray(X)
    B, S, _ = X.shape
    if S not in _CACHE:
        _CACHE[S] = build_kernel(S)
    nc = _CACHE[S]
    maps = prep_core_inputs(
        np.asarray(X),
        np.asarray(Wq),
        np.asarray(bq),
        np.asarray(Wk),
        np.asarray(bk),
        np.asarray(Wv),
        np.asarray(bv),
        np.asarray(Wo),
        np.asarray(bo),
        S,
    )
    res = run_bass_kernel_spmd(nc, maps, list(range(8)))
    out = np.zeros((B, S, D), dtype=np.float32)
    for c in range(8):
        out[c // 2] += res.results[c]["o"]
    out += np.asarray(bo)[None, None, :].astype(np.float32)
    return out
